# revision 13
# baseline (speedup 1.0000x reference)
"""BiMPM matching-layer kernel for Trainium2 (8 NeuronCores, pure data parallel).

Computes, per batch b and direction d (fw/bw, H=128 halves of the 256-dim inputs):
  m1: full matching vs last/first q timestep
  m2: max-pooling matching (max over q of per-perspective cosine)
  m3: mean-attentive matching (cosine-matrix weighted mean, scale-invariant form)
  m4: max-attentive matching (argmax over q of cosine matrix; gathers batch-0 q rows,
      faithful to the reference's flattened-index behavior)
Output (B, S1, 8*L) with L=20 perspectives.

Sharding: batch dim 64 -> 8 batches per core.  The 16 (b,d) pairs per core run
through a 3-deep software pipeline (emission order A(i), B(i-1), C(i-2)):
  A: input DMAs, squares, norm matmuls, fused rsqrt chains, rs2 DRAM-bounce
     broadcast, m1 numerators
  B: cosine-matrix paths (GT, mean-attentive GWT, argmax + index bounce), m1/m3
     sign prep, v2-replicate DMA for m2
  C: m2 (v2ws = replicated-v2 * w2rep * rs2 as two big fp16 TTs, 512-col fp16
     matmuls, all PSUM drains on ACT, fp16 TT max-tree split DVE/Pool),
     m4 mask/gather path, m3/m4 finals, output
so that bounce-DMA latencies and cross-engine chains overlap other pairs' work
and the PE never idles long enough to re-throttle (HAM).
"""
import numpy as np

B, S, H, L = 64, 256, 128, 20
NCORES = 8
BPC = B // NCORES
NP = 2 * BPC  # pairs per core
EPS = 1e-8

_cache = {}


def _build_bass():
    from contextlib import ExitStack

    import concourse.bass as bass
    import concourse.tile as tile
    from concourse import mybir

    f32 = mybir.dt.float32
    f16 = mybir.dt.float16
    bf16 = mybir.dt.bfloat16
    AF = mybir.ActivationFunctionType
    OP = mybir.AluOpType

    nc = bass.Bass()

    # DRAM I/O (per core)
    mrgF = nc.dram_tensor("mrgF", [BPC, H, 2, 768], f32, kind="ExternalInput")
    mrgH = nc.dram_tensor("mrgH", [BPC, H, 2, 512], f16, kind="ExternalInput")
    w2T = nc.dram_tensor("w2T", [2, H, 81], f32, kind="ExternalInput")
    w2repI = nc.dram_tensor("w2repI", [2, H, L * S], f16, kind="ExternalInput")
    q0nb = nc.dram_tensor("q0nb", [2, S, H], f16, kind="ExternalInput")
    iota2 = nc.dram_tensor("iota2", [H, 2], f32, kind="ExternalInput")
    onesr = nc.dram_tensor("onesr", [1, H], f32, kind="ExternalInput")
    ident = nc.dram_tensor("ident", [H, H], f32, kind="ExternalInput")
    out = nc.dram_tensor("out", [BPC, S, 8 * L], f32, kind="ExternalOutput")
    # DRAM bounces for row broadcasts (per-pair slots: no WAR hazards)
    r2scr = nc.dram_tensor("r2scr", [BPC, 2, 1, L * S], f16, kind="Internal")
    idxscr = nc.dram_tensor("idxscr", [BPC, 2, 1, S], f16, kind="Internal")

    with tile.TileContext(nc) as tc, ExitStack() as ctx:
        cons = ctx.enter_context(tc.tile_pool(name="cons", bufs=1))
        batchp = ctx.enter_context(tc.tile_pool(name="batchp", bufs=3))
        apool = ctx.enter_context(tc.tile_pool(name="apool", bufs=3))
        repp = ctx.enter_context(tc.tile_pool(name="repp", bufs=3))
        bpool = ctx.enter_context(tc.tile_pool(name="bpool", bufs=2))
        v2wsp = ctx.enter_context(tc.tile_pool(name="v2wsp", bufs=3))
        m2pool = ctx.enter_context(tc.tile_pool(name="m2pool", bufs=1))
        cpoolx = ctx.enter_context(tc.tile_pool(name="cpoolx", bufs=2))
        outp = ctx.enter_context(tc.tile_pool(name="outp", bufs=2))
        # PSUM: 8 banks = psNum 2x[H,1024](4) + psA 2x[H,512](2) + psB 2x[H,512](2)
        psNum_p = ctx.enter_context(tc.tile_pool(name="psNum", bufs=2, space="PSUM"))
        psA_p = ctx.enter_context(tc.tile_pool(name="psA", bufs=2, space="PSUM"))
        psB_p = ctx.enter_context(tc.tile_pool(name="psB", bufs=2, space="PSUM"))

        # ---- constants ----
        w2_sb = cons.tile([H, 2, 81], f32)
        nc.sync.dma_start(out=w2_sb, in_=w2T[:].rearrange("d h c -> h d c"))
        w2rep = cons.tile([H, 2, L, S], f16)
        nc.sync.dma_start(out=w2rep, in_=w2repI[:].rearrange("d h (l q) -> h d l q", l=L))
        q0cb = cons.tile([H, 2, 2, H], f16)
        nc.sync.dma_start(out=q0cb, in_=q0nb[:].rearrange("d (c q) h -> q d c h", c=2))
        iota_sb = cons.tile([H, 2], f32)
        nc.sync.dma_start(out=iota_sb, in_=iota2[:])
        onesr_sb = cons.tile([1, H], f32)
        nc.sync.dma_start(out=onesr_sb, in_=onesr[:])
        ident_sb = cons.tile([H, H], f32)
        nc.sync.dma_start(out=ident_sb, in_=ident[:])
        # Touch every const once so const-DMA waits are absorbed here and later
        # instructions don't re-carry them.
        warm = cons.tile([H, 8], f32)
        nc.vector.tensor_copy(warm[:, 0:1], w2_sb[:, 0, 0:1])
        nc.vector.tensor_copy(warm[:, 1:2].bitcast(f16)[:, 0:1], w2rep[:, 0, 0, 0:1])
        nc.vector.tensor_copy(warm[:, 2:3].bitcast(f16)[:, 0:1], q0cb[:, 0, 0, 0:1])
        nc.vector.tensor_copy(warm[:, 3:4], iota_sb[:, 0:1])
        nc.vector.tensor_copy(warm[0:1, 4:5], onesr_sb[0:1, 0:1])
        nc.vector.tensor_copy(warm[:, 5:6], ident_sb[:, 0:1])
        ab_w2 = nc.tensor.ldweights(w2_sb[:, 0, 0:1].bitcast(bf16))
        ab_q0 = nc.tensor.ldweights(q0cb[:, 0, 0, 0:1])
        ab_id = nc.tensor.ldweights(ident_sb[:, 0:1].bitcast(bf16))
        bass._add_dep_helper(ab_q0.ins, ab_w2.ins, sync=False, reason="const absorb chain")
        bass._add_dep_helper(ab_id.ins, ab_q0.ins, sync=False, reason="const absorb chain")

        def dep(from_inst, to_inst, why="absorb order"):
            bass._add_dep_helper(from_inst.ins, to_inst.ins, sync=False, reason=why)

        def pe_tickle():
            # dep-free 1-col weight load: keeps the PE HAM activity window
            # busy through short idle stretches so the clock stays at 2.4GHz
            nc.tensor.ldweights(w2_sb[:, 0, 0:1].bitcast(bf16))

        # per-pair state carried between phases
        st = [dict() for _ in range(NP)]
        batch_st = [dict() for _ in range(BPC)]
        last_pe_absorb = [ab_id]

        def emit_A(i):
            b, d = i // 2, i % 2
            first = i == 0
            if d == 0:
                bF = batchp.tile([H, 2, 768], f32, name="bF")
                nc.sync.dma_start(out=bF, in_=mrgF[b])
                bH = batchp.tile([H, 2, 512], f16, name="bH")
                nc.sync.dma_start(out=bH, in_=mrgH[b])
                outt0 = outp.tile([H, 8 * L], f32, name="outt0")
                outt1 = outp.tile([H, 8 * L], f32, name="outt1")
                batch_st[b] = dict(bF=bF, bH=bH, outts=(outt0, outt1))
                # absorb the batch-load DMA sems once on DVE and PE
                scr = apool.tile([H, 2], f32, name="dve_scr")
                nc.vector.tensor_copy(scr[:, 0:1], bF[:, 0, 0:1])
                nc.vector.tensor_copy(scr[:, 1:2].bitcast(f16)[:, 0:1], bH[:, 0, 0:1])
                abF = nc.tensor.ldweights(bF[:, 0, 0:1].bitcast(bf16))
                abH = nc.tensor.ldweights(bH[:, 0, 0:1])
                dep(abF, last_pe_absorb[0], "pe absorb chain")
                dep(abH, abF, "pe absorb chain")
                last_pe_absorb[0] = abH
            bs = batch_st[b]
            bF, bH = bs["bF"], bs["bH"]
            v1Tf = bF[:, d, 0:256]
            v2Tf = bF[:, d, 256:512]
            w2d = w2_sb[:, d, :]

            # squares of v1|v2 in one Pool op
            sq = apool.tile([H, 512], f32, name="sq")
            nc.gpsimd.tensor_tensor(sq, bF[:, d, 0:512], bF[:, d, 0:512], op=OP.mult)

            pa = psA_p.tile([H, 512], f32, name="pa")
            # [0:80 n1-c0 | 80:160 n1-c1 | 160:162 v1ones (c0,c1) | 162:164 v2ones
            #  | 164:184 n2a-rep | 184:204 n2a row (p0) | 204:244 num1 (c0,c1)
            #  | 244:246 Gr | n2bT at [0:20,256:512] | idxT at [0:1,256:512] later ]
            mms = [
                nc.tensor.matmul(pa[:, 0:80], sq[:, 0:H], w2d[:, 0:80], start=True, stop=True),
                nc.tensor.matmul(pa[:, 80:160], sq[:, H:S], w2d[:, 0:80], start=True, stop=True),
                nc.tensor.matmul(pa[:, 160:161], sq[:, 0:H], w2d[:, 80:81], start=True, stop=True),
                nc.tensor.matmul(pa[:, 161:162], sq[:, H:S], w2d[:, 80:81], start=True, stop=True),
                nc.tensor.matmul(pa[:, 162:163], sq[:, 256:384], w2d[:, 80:81], start=True, stop=True),
                nc.tensor.matmul(pa[:, 163:164], sq[:, 384:512], w2d[:, 80:81], start=True, stop=True),
                nc.tensor.matmul(pa[0:20, 256:512], w2d[:, 20:40], sq[:, 256:512], start=True, stop=True),
            ]
            if first:
                for mm in mms:
                    dep(mm, ab_id, "consts before first mms")

            # m1 numerators (before the fused rsqrt: n2a-rep joins rsa's recip)
            tcol = v2Tf[:, 255:256] if d == 0 else v2Tf[:, 0:1]
            sqt = apool.tile([H, 1], f32, name="sqt")
            nc.vector.tensor_tensor(sqt, tcol, tcol, op=OP.mult)
            rhs1 = apool.tile([H, 20], f32, name="rhs1")
            nc.vector.tensor_scalar_mul(rhs1, w2d[:, 0:20], tcol)
            nc.tensor.matmul(pa[0:1, 184:204], sqt, w2d[:, 0:20], start=True, stop=True)
            n2a_sb = apool.tile([1, 20], f32, name="n2a_sb")
            nc.scalar.copy(n2a_sb, pa[0:1, 184:204])
            nc.tensor.matmul(pa[:, 164:184], onesr_sb, n2a_sb, start=True, stop=True)
            nc.tensor.matmul(pa[:, 204:224], v1Tf[:, 0:H], rhs1, start=True, stop=True)
            nc.tensor.matmul(pa[:, 224:244], v1Tf[:, H:S], rhs1, start=True, stop=True)

            # fused rsqrt of n1 + ones-norms + rs2rep: one reciprocal + one sqrt
            rsa = apool.tile([H, 184], f32, name="rsa")
            nc.vector.reciprocal(rsa, pa[:, 0:184])
            nc.scalar.sqrt(rsa, rsa)
            # rs2 for m2: [20,256] rsqrt -> f16 -> DRAM bounce broadcast
            r2bT = apool.tile([20, 256], f32, name="r2bT")
            nc.vector.reciprocal(r2bT, pa[0:20, 256:512])
            r2bTb = apool.tile([20, 256], f16, name="r2bTb")
            nc.scalar.sqrt(r2bTb, r2bT)
            nc.sync.dma_start(
                out=r2scr[b, d].rearrange("o (l s) -> (o l) s", l=L), in_=r2bTb[:]
            )
            repb = repp.tile([H, L, S], f16, name="repb")
            nc.sync.dma_start(
                out=repb[:].rearrange("h l s -> h (l s)"),
                in_=r2scr[b, d].to_broadcast((H, L * S)),
            )

            # m2 stream prep: v2wsb = v2 * w2 per perspective, on the (idle)
            # Pool engine via stride-0-broadcast TTs; *rs2 happens in C on DVE.
            # Emitted 2 rounds ahead of use so the Pool WAR on pair i-3's m2
            # matmuls never stalls the pipeline.
            v2wsb = v2wsp.tile([H, L, S], f16, name="v2wsb")
            v2Tb_b = bH[:, d, 256:512].rearrange("h q -> h () q").to_broadcast((H, 4, S))
            for j in range(5):
                nc.gpsimd.tensor_tensor(
                    v2wsb[:, 4 * j : 4 * j + 4, :], v2Tb_b, w2rep[:, d, 4 * j : 4 * j + 4, :], op=OP.mult
                )
            pe_tickle()

            # eps * ||v1|| per chunk (for the m3 sign), both chunks at once
            nv1e = apool.tile([H, 2], f32, name="nv1e")
            nc.scalar.sqrt(nv1e, pa[:, 160:162])
            nc.scalar.mul(nv1e, nv1e, EPS)

            st[i].update(
                pa=pa, rsa=rsa, repb=repb, nv1e=nv1e, v2wsb=v2wsb,
                v1Tf=v1Tf, v2Tf=v2Tf,
                v1Tb=bH[:, d, 0:256], v2Tb=bH[:, d, 256:512],
                v2nat=bF[:, d, 512:768].rearrange("p (c h) -> p c h", c=2),
                outts=bs["outts"], b=b, d=d,
            )

        def emit_B(i):
            s = st[i]
            b, d = s["b"], s["d"]
            pa, rsa = s["pa"], s["rsa"]
            v1Tf, v2Tf, v2nat = s["v1Tf"], s["v2Tf"], s["v2nat"]
            outts = s["outts"]

            pb = psB_p.tile([H, 512], f32, name="pb")
            # GT[q, (c p)] = v2^T v1
            nc.tensor.matmul(pb[:, 0:256], v2Tf[:, 0:H], v1Tf, start=True, stop=True)
            nc.tensor.matmul(pb[:, 256:512], v2Tf[:, H:S], v1Tf, start=True, stop=True)
            GT_sb = bpool.tile([H, 2, S], f32, name="GT_sb")
            nc.scalar.copy(GT_sb[:].rearrange("h c q -> h (c q)"), pb[:, 0:512])
            pe_tickle()

            # v2r = v2 rows * rs2 (per-q-partition scale on ACT)
            v2r = bpool.tile([H, 2, H], f32, name="v2r")
            nc.scalar.activation(v2r[:, 0, :], v2nat[:, 0, :], AF.Copy, scale=rsa[:, 162:163])
            nc.scalar.activation(v2r[:, 1, :], v2nat[:, 1, :], AF.Copy, scale=rsa[:, 163:164])

            # GWT (mean-attentive, scale-invariant) at pb[0:256]; v2r^T at [256:512]
            nc.tensor.matmul(pb[:, 0:256], v2r[:, 0, :], GT_sb[:, 0, :], start=True, stop=False)
            nc.tensor.matmul(pb[:, 0:256], v2r[:, 1, :], GT_sb[:, 1, :], start=False, stop=True)
            tr0 = nc.tensor.transpose(pb[:, 256:384], v2r[:, 0, :], ident_sb)
            tr1 = nc.tensor.transpose(pb[:, 384:512], v2r[:, 1, :], ident_sb)
            if i == 0:
                dep(tr0, ab_id, "ident absorbed before transpose")
                dep(tr1, ab_id, "ident absorbed before transpose")
            v2nT = bpool.tile([H, S], f32, name="v2nT")
            nc.scalar.copy(v2nT, pb[:, 256:512])
            prod3 = bpool.tile([H, S], f32, name="prod3")
            nc.vector.tensor_tensor(prod3, v1Tf, pb[:, 0:256], op=OP.mult)
            sq3 = bpool.tile([H, S], f32, name="sq3")
            nc.scalar.square(sq3, pb[:, 0:256])

            # G' = v1 . v2n for argmax (rs1[p] scale drops out)
            nc.tensor.matmul(pb[:, 0:256], v1Tf[:, 0:H], v2nT, start=True, stop=True)
            nc.tensor.matmul(pb[:, 256:512], v1Tf[:, H:S], v2nT, start=True, stop=True)
            idxf = bpool.tile([H, 2], f32, name="idxf")
            idx8 = bpool.tile([H, 2, 8], mybir.dt.uint32, name="idx8")
            for c in range(2):
                top8 = bpool.tile([H, 8], f32, name="top8")
                nc.vector.max_with_indices(top8, idx8[:, c, :], pb[:, 256 * c : 256 * c + 256])
            nc.vector.tensor_copy(idxf, idx8[:, :, 0:1].rearrange("p c o -> p (c o)"))
            for c in range(2):
                tr = nc.tensor.transpose(
                    pa[0:1, 256 + c * H : 256 + c * H + H], idxf[:, c : c + 1], ident_sb
                )
                if i == 0:
                    dep(tr, ab_id, "ident absorbed before transpose")
            idxT_sb = bpool.tile([1, 256], f16, name="idxT_sb")
            nc.scalar.copy(idxT_sb, pa[0:1, 256:512])
            nc.sync.dma_start(out=idxscr[b, d], in_=idxT_sb[:])
            idxrepb = bpool.tile([H, 256], f16, name="idxrepb")
            nc.sync.dma_start(out=idxrepb, in_=idxscr[b, d].to_broadcast((H, S)))
            pe_tickle()

            # m3 sign: Gr = sum_q num[q,p]*rs2[q], sgn = Sign(Gr + eps||v1||)
            for c in range(2):
                nc.tensor.matmul(pa[:, 244 + c : 245 + c], GT_sb[:, 0, c * H : c * H + H],
                                 rsa[:, 162:163], start=True, stop=False)
                nc.tensor.matmul(pa[:, 244 + c : 245 + c], GT_sb[:, 1, c * H : c * H + H],
                                 rsa[:, 163:164], start=False, stop=True)
            sgn = bpool.tile([H, 2], f32, name="sgn")
            nc.scalar.activation(sgn[:, 0:1], pa[:, 244:245], AF.Sign, bias=s["nv1e"][:, 0:1], scale=1.0)
            nc.scalar.activation(sgn[:, 1:2], pa[:, 245:246], AF.Sign, bias=s["nv1e"][:, 1:2], scale=1.0)

            # m1 finals
            t1b = bpool.tile([H, 2, 20], f32, name="t1b")
            nc.vector.tensor_tensor(
                t1b,
                pa[:, 204:244].rearrange("p (c x) -> p c x", c=2),
                rsa[:, 0:160].rearrange("p (c x) -> p c x", c=2)[:, :, 0:20],
                op=OP.mult,
            )
            for c in range(2):
                nc.vector.tensor_tensor(
                    outts[c][:, d * 20 : d * 20 + 20], t1b[:, c, :], rsa[:, 164:184], op=OP.mult
                )
            s.update(GT_sb=GT_sb, prod3=prod3, sq3=sq3, idxrepb=idxrepb, sgn=sgn)

        def emit_C(i):
            s = st[i]
            b, d = s["b"], s["d"]
            pa, rsa, repb = s["pa"], s["rsa"], s["repb"]
            v1Tf, v1Tb = s["v1Tf"], s["v1Tb"]
            outts = s["outts"]
            w2d = w2_sb[:, d, :]

            # masks from the (landed) idx broadcast, then att4 gather + m3/m4 nums
            maskT0 = cpoolx.tile([H, 256], f16, name="maskT0")
            nc.vector.tensor_scalar(maskT0, s["idxrepb"], iota_sb[:, 0:1], None, op0=OP.is_equal)
            maskT1 = cpoolx.tile([H, 256], f16, name="maskT1")
            nc.vector.tensor_scalar(maskT1, s["idxrepb"], iota_sb[:, 1:2], None, op0=OP.is_equal)

            pc0 = psNum_p.tile([H, 4, 256], f32, name="psNum")
            pc0f = pc0[:].rearrange("p l q -> p (l q)")
            a4_mm0 = nc.tensor.matmul(pc0f[:, 160:416], q0cb[:, d, 0, :], maskT0, start=True, stop=False)
            a4_mm1 = nc.tensor.matmul(pc0f[:, 160:416], q0cb[:, d, 1, :], maskT1, start=False, stop=True)
            if i == 0:
                dep(a4_mm0, ab_id, "q0c absorbed before att4T")
                dep(a4_mm1, ab_id, "q0c absorbed before att4T")
            # pc0 layout: [0:40 num3 (c0,c1) | 40:80 num4 | 80:120 n3 | 120:160 n4
            #              | 160:416 att4T]
            prod3, sq3 = s["prod3"], s["sq3"]
            for c in range(2):
                sl = slice(c * H, c * H + H)
                nc.tensor.matmul(pc0f[:, c * 20 : c * 20 + 20], prod3[:, sl], w2d[:, 40:60], start=True, stop=True)
                nc.tensor.matmul(pc0f[:, 80 + c * 20 : 80 + c * 20 + 20], sq3[:, sl], w2d[:, 40:60], start=True, stop=True)
            prod4 = cpoolx.tile([H, S], f32, name="prod4")
            nc.vector.tensor_tensor(prod4, v1Tf, pc0f[:, 160:416], op=OP.mult)
            sq4 = cpoolx.tile([H, S], f32, name="sq4")
            nc.scalar.square(sq4, pc0f[:, 160:416])
            for c in range(2):
                sl = slice(c * H, c * H + H)
                nc.tensor.matmul(pc0f[:, 40 + c * 20 : 40 + c * 20 + 20], prod4[:, sl], w2d[:, 60:80], start=True, stop=True)
                nc.tensor.matmul(pc0f[:, 120 + c * 20 : 120 + c * 20 + 20], sq4[:, sl], w2d[:, 60:80], start=True, stop=True)

            # m3/m4 finals
            rsq34 = cpoolx.tile([H, 80], f32, name="rsq34")
            nc.vector.reciprocal(rsq34, pc0f[:, 80:160])
            nc.scalar.sqrt(rsq34, rsq34)
            t34 = cpoolx.tile([H, 2, 2, 20], f32, name="t34")  # [m34, c, l]
            for j in range(2):  # 0: m3 (w5/w6 rs1), 1: m4 (w7/w8 rs1)
                nc.vector.tensor_tensor(
                    t34[:, j],
                    pc0f[:, j * 40 : j * 40 + 40].rearrange("p (c x) -> p c x", c=2),
                    rsa[:, 0:160].rearrange("p (c x) -> p c x", c=2)[:, :, 40 + 20 * j : 60 + 20 * j],
                    op=OP.mult,
                )
            t34b = cpoolx.tile([H, 2, 2, 20], f32, name="t34b")
            nc.vector.tensor_tensor(
                t34b, t34,
                rsq34[:].rearrange("p (j c x) -> p j c x", j=2, c=2),
                op=OP.mult,
            )
            for c in range(2):
                nc.scalar.mul(
                    outts[c][:, 80 + d * 20 : 80 + d * 20 + 20], t34b[:, 0, c, :], s["sgn"][:, c : c + 1]
                )
                nc.vector.tensor_copy(
                    outts[c][:, 120 + d * 20 : 120 + d * 20 + 20], t34b[:, 1, c, :]
                )

            # ---- m2 ----
            # v2ws = (v2*w2, Pool-built in B) * rs2, one in-place fp16 TT
            v2wsb = s["v2wsb"]
            vwf = v2wsb[:].rearrange("h l q -> h (l q)")
            nc.vector.tensor_tensor(vwf, vwf, repb[:].rearrange("h l s -> h (l s)"), op=OP.mult)

            stage = m2pool.tile([H, 2, L, S], f16, name="stage")
            trt = m2pool.tile([H, 2, L, H], f16, name="trt")
            for c in range(2):
                for j in range(5):
                    pc = psNum_p.tile([H, 4, 256], f32, name="psNum")
                    pcf = pc[:].rearrange("p l q -> p (l q)")
                    mm = nc.tensor.matmul(
                        pcf[:, 0:512], v1Tb[:, c * H : c * H + H], vwf[:, 1024 * j : 1024 * j + 512],
                        start=True, stop=True,
                    )
                    nc.tensor.matmul(
                        pcf[:, 512:1024], v1Tb[:, c * H : c * H + H], vwf[:, 1024 * j + 512 : 1024 * j + 1024],
                        start=True, stop=True,
                    )
                    if i == 0 and c == 0 and j == 0:
                        dep(mm, last_pe_absorb[0], "absorbs before m2")
                    nc.scalar.copy(stage[:, c, 4 * j : 4 * j + 4, :], pc[:])
                    pe_tickle()
                # tree level 1 for this chunk right away, so the next pair's
                # drains (stage WAR, bufs=1) unblock as early as possible
                nc.vector.tensor_tensor(trt[:, c], stage[:, c, :, 0:H], stage[:, c, :, H:S], op=OP.max)

            # fp16 TT max-tree levels 2+: ping-pong trt/tr2 down to m2pre
            tr2 = m2pool.tile([H, 2, L, 64], f16, name="tr2")
            nc.vector.tensor_tensor(tr2, trt[:, :, :, 0:64], trt[:, :, :, 64:128], op=OP.max)
            nc.vector.tensor_tensor(trt[:, :, :, 0:32], tr2[:, :, :, 0:32], tr2[:, :, :, 32:64], op=OP.max)
            nc.vector.tensor_tensor(tr2[:, :, :, 0:16], trt[:, :, :, 0:16], trt[:, :, :, 16:32], op=OP.max)
            nc.vector.tensor_tensor(trt[:, :, :, 0:8], tr2[:, :, :, 0:8], tr2[:, :, :, 8:16], op=OP.max)
            nc.vector.tensor_tensor(tr2[:, :, :, 0:4], trt[:, :, :, 0:4], trt[:, :, :, 4:8], op=OP.max)
            nc.vector.tensor_tensor(trt[:, :, :, 0:2], tr2[:, :, :, 0:2], tr2[:, :, :, 2:4], op=OP.max)
            m2pre = cpoolx.tile([H, 2, L], f32, name="m2pre")
            nc.vector.tensor_tensor(
                m2pre,
                trt[:, :, :, 0:1].rearrange("p c l o -> p c (l o)"),
                trt[:, :, :, 1:2].rearrange("p c l o -> p c (l o)"),
                op=OP.max,
            )
            for c in range(2):
                nc.vector.tensor_tensor(
                    outts[c][:, 40 + d * 20 : 40 + d * 20 + 20],
                    m2pre[:, c, :],
                    rsa[:, 80 * c + 20 : 80 * c + 40],
                    op=OP.mult,
                )
            if d == 1:
                nc.sync.dma_start(out=out[b, 0:H, :], in_=outts[0])
                nc.sync.dma_start(out=out[b, H:S, :], in_=outts[1])

        for r in range(NP + 2):
            if r < NP:
                emit_A(r)
            if r >= 2:
                emit_C(r - 2)
            if 1 <= r <= NP:
                emit_B(r - 1)

    return nc


def _prep_core_inputs(p, q, w_list, core):
    """Host-side layout prep for one core. Only layout transforms + weight-only math."""
    sl = slice(core * BPC, (core + 1) * BPC)
    p8 = np.ascontiguousarray(p[sl])  # (BPC, 256, 256)
    q8 = np.ascontiguousarray(q[sl])
    # [b, h, d, 0:256]=pT, [256:512]=qT, [512:768]=qn rows (c,h)
    pT = p8.reshape(BPC, S, 2, H).transpose(0, 3, 2, 1)  # (BPC, H, 2, S)
    qT = q8.reshape(BPC, S, 2, H).transpose(0, 3, 2, 1)
    # qn[b, qp, d, c, h] = q8[b, c*128+qp, d*H+h]
    qn = q8.reshape(BPC, 2, H, 2, H).transpose(0, 2, 3, 1, 4)  # (BPC, qp, d, c, h)
    mrgF = np.empty((BPC, H, 2, 768), np.float32)
    mrgF[..., 0:256] = pT
    mrgF[..., 256:512] = qT
    mrgF[..., 512:768] = qn.reshape(BPC, H, 2, 256)
    mrgH = np.empty((BPC, H, 2, 512), np.float16)
    mrgH[..., 0:256] = pT
    mrgH[..., 256:512] = qT

    q0n = np.ascontiguousarray(q[0].reshape(S, 2, H).transpose(1, 0, 2))  # (2, S, H)

    w2T = np.empty((2, H, 81), np.float32)
    for d in range(2):
        ws = w_list[d::2]  # fw: w1,w3,w5,w7 ; bw: w2,w4,w6,w8
        cat = np.concatenate([w * w for w in ws] + [np.ones((1, H), np.float32)], 0)
        w2T[d] = cat.T
    # w2rep[d, h, l*S+s] = (w3/w4)^2[l, h] replicated over s
    w2rep = np.empty((2, H, L * S), np.float16)
    for d in range(2):
        w2 = (w_list[2 + d].astype(np.float32) ** 2).astype(np.float16)  # (L, H)
        w2rep[d] = np.repeat(w2.T[:, :, None], S, axis=2).reshape(H, L * S)
    iota2 = np.stack([np.arange(H, dtype=np.float32), np.arange(H, 2 * H, dtype=np.float32)], 1)

    return {
        "mrgF": mrgF,
        "mrgH": mrgH,
        "w2T": w2T,
        "w2repI": w2rep,
        "q0nb": q0n.astype(np.float16),
        "iota2": np.ascontiguousarray(iota2),
        "onesr": np.ones((1, H), np.float32),
        "ident": np.eye(H, dtype=np.float32),
    }


def _legalize_bir(bir_bytes):
    """This walrus build rejects >1 sync-wait command per instruction; move all
    but one wait of each instruction onto an inserted same-engine Drain."""
    import json as _json

    d = _json.loads(bir_bytes)
    n = 0
    for fnd in d["functions"]:
        for blk in fnd["blocks"]:
            insts = blk.get("instructions") or []
            out = []
            for ins in insts:
                si = ins.get("sync_info") or {}
                w = si.get("on_wait") or []
                if len(w) > 1:
                    for extra in w[:-1]:
                        out.append(
                            {
                                "debug": ins.get("debug", 0),
                                "engine": ins.get("engine"),
                                "ins": [],
                                "outs": [],
                                "is_reset_sema": False,
                                "name": f"I-legalw-{n}",
                                "opcode": "Drain",
                                "sync_info": {"on_update": [], "on_wait": [extra]},
                            }
                        )
                        n += 1
                    si["on_wait"] = [w[-1]]
                out.append(ins)
            blk["instructions"] = out
    return _json.dumps(d).encode(), n


def _install_legalizer():
    if _cache.get("legalizer"):
        return
    from concourse import bass2jax, bass_utils

    orig = bass_utils.compile_bir_kernel

    def patched(bir_json, tmpdir, neff_name="file.neff"):
        fixed, n = _legalize_bir(bir_json)
        return orig(fixed, tmpdir, neff_name)

    bass2jax.compile_bir_kernel = patched
    _cache["legalizer"] = True


def _get_runner():
    """Build the 8-core shard_map'd PJRT callable once."""
    if "runner" in _cache:
        return _cache["runner"]

    import jax
    from jax.sharding import Mesh, PartitionSpec
    from jax.experimental.shard_map import shard_map

    import concourse.mybir as mybir
    from concourse import bass2jax

    if "nc" not in _cache:
        _cache["nc"] = _build_bass()
    nc = _cache["nc"]

    bass2jax.install_neuronx_cc_hook()
    _install_legalizer()
    assert nc.dbg_addr is None
    partition_name = nc.partition_id_tensor.name if nc.partition_id_tensor else None

    in_names, out_names, out_avals, zero_outs = [], [], [], []
    for alloc in nc.m.functions[0].allocations:
        if not isinstance(alloc, mybir.MemoryLocationSet):
            continue
        name = alloc.memorylocations[0].name
        if alloc.kind == "ExternalInput":
            if name != partition_name:
                in_names.append(name)
        elif alloc.kind == "ExternalOutput":
            out_names.append(name)
            shape = tuple(alloc.tensor_shape)
            dtype = mybir.dt.np(alloc.dtype)
            out_avals.append(jax.core.ShapedArray(shape, dtype))
            zero_outs.append(np.zeros(shape, dtype))
    n_params = len(in_names)
    n_outs = len(out_avals)
    all_names = in_names + out_names
    if partition_name is not None:
        all_names = all_names + [partition_name]

    def _body(*args):
        operands = list(args)
        if partition_name is not None:
            operands.append(bass2jax.partition_id_tensor())
        outs = bass2jax._bass_exec_p.bind(
            *operands,
            out_avals=tuple(out_avals),
            in_names=tuple(all_names),
            out_names=tuple(out_names),
            lowering_input_output_aliases=(),
            sim_require_finite=True,
            sim_require_nnan=True,
            nc=nc,
        )
        return tuple(outs)

    devices = jax.devices()[:NCORES]
    mesh = Mesh(np.asarray(devices), ("core",))
    sharded = jax.jit(
        shard_map(
            _body,
            mesh=mesh,
            in_specs=(PartitionSpec("core"),) * (n_params + n_outs),
            out_specs=(PartitionSpec("core"),) * n_outs,
            check_rep=False,
        ),
        donate_argnums=tuple(range(n_params, n_params + n_outs)),
        keep_unused=True,
    )
    runner = {
        "jax": jax,
        "sharded": sharded,
        "mesh": mesh,
        "in_names": in_names,
        "out_names": out_names,
        "out_avals": out_avals,
        "zero_outs": zero_outs,
        "n_params": n_params,
    }
    _cache["runner"] = runner
    return runner


def kernel(p, q, w1, w2, w3, w4, w5, w6, w7, w8, _time_iters=0):
    p = np.asarray(p, dtype=np.float32)
    q = np.asarray(q, dtype=np.float32)
    w_list = [np.asarray(w, dtype=np.float32) for w in (w1, w2, w3, w4, w5, w6, w7, w8)]

    r = _get_runner()
    jax = r["jax"]
    in_maps = [_prep_core_inputs(p, q, w_list, c) for c in range(NCORES)]
    concat_in = [
        np.concatenate([in_maps[c][name] for c in range(NCORES)], 0)
        for name in r["in_names"]
    ]
    concat_zeros = [
        np.zeros((NCORES * z.shape[0], *z.shape[1:]), z.dtype) for z in r["zero_outs"]
    ]
    out_arrs = r["sharded"](*concat_in, *concat_zeros)
    jax.block_until_ready(out_arrs)
    out = np.asarray(out_arrs[r["out_names"].index("out")])  # (64, 256, 160)

    if _time_iters:
        import time

        from jax.sharding import NamedSharding, PartitionSpec

        shd = NamedSharding(r["mesh"], PartitionSpec("core"))
        dev_in = [jax.device_put(a, shd) for a in concat_in]
        jax.block_until_ready(dev_in)
        times = []
        for _ in range(_time_iters):
            zeros = [
                jax.device_put(np.zeros((NCORES * z.shape[0], *z.shape[1:]), z.dtype), shd)
                for z in r["zero_outs"]
            ]
            jax.block_until_ready(zeros)
            t0 = time.perf_counter()
            o = r["sharded"](*dev_in, *zeros)
            jax.block_until_ready(o)
            times.append(time.perf_counter() - t0)
        kernel.last_exec_time_ns = int(min(times) * 1e9)
        kernel.all_times_ns = [int(t * 1e9) for t in times]
    return out


# revision 14
# speedup vs baseline: 1.0128x; 1.0128x over previous
"""BiMPM matching-layer kernel for Trainium2 (8 NeuronCores, pure data parallel).

Computes, per batch b and direction d (fw/bw, H=128 halves of the 256-dim inputs):
  m1: full matching vs last/first q timestep
  m2: max-pooling matching (max over q of per-perspective cosine)
  m3: mean-attentive matching (cosine-matrix weighted mean, scale-invariant form)
  m4: max-attentive matching (argmax over q of cosine matrix; gathers batch-0 q rows,
      faithful to the reference's flattened-index behavior)
Output (B, S1, 8*L) with L=20 perspectives.

Sharding: batch dim 64 -> 8 batches per core.  The 16 (b,d) pairs per core run
through a 3-deep software pipeline (emission order A(i), B(i-1), C(i-2)):
  A: input DMAs, squares, norm matmuls, fused rsqrt chains, rs2 DRAM-bounce
     broadcast, m1 numerators
  B: cosine-matrix paths (GT, mean-attentive GWT, argmax + index bounce), m1/m3
     sign prep, v2-replicate DMA for m2
  C: m2 (v2ws = replicated-v2 * w2rep * rs2 as two big fp16 TTs, 512-col fp16
     matmuls, all PSUM drains on ACT, fp16 TT max-tree split DVE/Pool),
     m4 mask/gather path, m3/m4 finals, output
so that bounce-DMA latencies and cross-engine chains overlap other pairs' work
and the PE never idles long enough to re-throttle (HAM).
"""
import numpy as np

B, S, H, L = 64, 256, 128, 20
NCORES = 8
BPC = B // NCORES
NP = 2 * BPC  # pairs per core
EPS = 1e-8

_cache = {}


def _build_bass():
    from contextlib import ExitStack

    import concourse.bass as bass
    import concourse.tile as tile
    from concourse import mybir

    f32 = mybir.dt.float32
    f16 = mybir.dt.float16
    bf16 = mybir.dt.bfloat16
    AF = mybir.ActivationFunctionType
    OP = mybir.AluOpType

    nc = bass.Bass()

    # DRAM I/O (per core)
    mrgF = nc.dram_tensor("mrgF", [BPC, H, 2, 768], f32, kind="ExternalInput")
    mrgH = nc.dram_tensor("mrgH", [BPC, H, 2, 512], f16, kind="ExternalInput")
    w2T = nc.dram_tensor("w2T", [2, H, 81], f32, kind="ExternalInput")
    w2repI = nc.dram_tensor("w2repI", [2, H, L * S], f16, kind="ExternalInput")
    q0nb = nc.dram_tensor("q0nb", [2, S, H], f16, kind="ExternalInput")
    iota2 = nc.dram_tensor("iota2", [H, 2], f32, kind="ExternalInput")
    onesr = nc.dram_tensor("onesr", [1, H], f32, kind="ExternalInput")
    ident = nc.dram_tensor("ident", [H, H], f32, kind="ExternalInput")
    out = nc.dram_tensor("out", [BPC, S, 8 * L], f32, kind="ExternalOutput")
    # DRAM bounces for row broadcasts (per-pair slots: no WAR hazards)
    r2scr = nc.dram_tensor("r2scr", [BPC, 2, 1, L * S], f16, kind="Internal")
    idxscr = nc.dram_tensor("idxscr", [BPC, 2, 1, S], f16, kind="Internal")

    with tile.TileContext(nc) as tc, ExitStack() as ctx:
        cons = ctx.enter_context(tc.tile_pool(name="cons", bufs=1))
        batchp = ctx.enter_context(tc.tile_pool(name="batchp", bufs=3))
        apool = ctx.enter_context(tc.tile_pool(name="apool", bufs=3))
        repp = ctx.enter_context(tc.tile_pool(name="repp", bufs=3))
        bpool = ctx.enter_context(tc.tile_pool(name="bpool", bufs=2))
        v2wsp = ctx.enter_context(tc.tile_pool(name="v2wsp", bufs=3))
        m2pool = ctx.enter_context(tc.tile_pool(name="m2pool", bufs=1))
        cpoolx = ctx.enter_context(tc.tile_pool(name="cpoolx", bufs=2))
        outp = ctx.enter_context(tc.tile_pool(name="outp", bufs=2))
        # PSUM: 8 banks = psNum 2x[H,1024](4) + psA 2x[H,512](2) + psB 2x[H,512](2)
        psNum_p = ctx.enter_context(tc.tile_pool(name="psNum", bufs=2, space="PSUM"))
        psA_p = ctx.enter_context(tc.tile_pool(name="psA", bufs=2, space="PSUM"))
        psB_p = ctx.enter_context(tc.tile_pool(name="psB", bufs=2, space="PSUM"))

        # ---- constants ----
        w2_sb = cons.tile([H, 2, 81], f32)
        nc.sync.dma_start(out=w2_sb, in_=w2T[:].rearrange("d h c -> h d c"))
        w2rep = cons.tile([H, 2, L, S], f16)
        nc.sync.dma_start(out=w2rep, in_=w2repI[:].rearrange("d h (l q) -> h d l q", l=L))
        q0cb = cons.tile([H, 2, 2, H], f16)
        nc.sync.dma_start(out=q0cb, in_=q0nb[:].rearrange("d (c q) h -> q d c h", c=2))
        iota_sb = cons.tile([H, 2], f32)
        nc.sync.dma_start(out=iota_sb, in_=iota2[:])
        onesr_sb = cons.tile([1, H], f32)
        nc.sync.dma_start(out=onesr_sb, in_=onesr[:])
        ident_sb = cons.tile([H, H], f32)
        nc.sync.dma_start(out=ident_sb, in_=ident[:])
        # Touch every const once so const-DMA waits are absorbed here and later
        # instructions don't re-carry them.
        warm = cons.tile([H, 8], f32)
        nc.vector.tensor_copy(warm[:, 0:1], w2_sb[:, 0, 0:1])
        nc.vector.tensor_copy(warm[:, 1:2].bitcast(f16)[:, 0:1], w2rep[:, 0, 0, 0:1])
        nc.vector.tensor_copy(warm[:, 2:3].bitcast(f16)[:, 0:1], q0cb[:, 0, 0, 0:1])
        nc.vector.tensor_copy(warm[:, 3:4], iota_sb[:, 0:1])
        nc.vector.tensor_copy(warm[0:1, 4:5], onesr_sb[0:1, 0:1])
        nc.vector.tensor_copy(warm[:, 5:6], ident_sb[:, 0:1])
        ab_w2 = nc.tensor.ldweights(w2_sb[:, 0, 0:1].bitcast(bf16))
        ab_q0 = nc.tensor.ldweights(q0cb[:, 0, 0, 0:1])
        ab_id = nc.tensor.ldweights(ident_sb[:, 0:1].bitcast(bf16))
        bass._add_dep_helper(ab_q0.ins, ab_w2.ins, sync=False, reason="const absorb chain")
        bass._add_dep_helper(ab_id.ins, ab_q0.ins, sync=False, reason="const absorb chain")

        def dep(from_inst, to_inst, why="absorb order"):
            bass._add_dep_helper(from_inst.ins, to_inst.ins, sync=False, reason=why)

        def pe_tickle():
            # dep-free 1-col weight load: keeps the PE HAM activity window
            # busy through short idle stretches so the clock stays at 2.4GHz
            nc.tensor.ldweights(w2_sb[:, 0, 0:1].bitcast(bf16))

        # per-pair state carried between phases
        st = [dict() for _ in range(NP)]
        batch_st = [dict() for _ in range(BPC)]
        last_pe_absorb = [ab_id]

        def emit_A(i):
            b, d = i // 2, i % 2
            first = i == 0
            if d == 0:
                bF = batchp.tile([H, 2, 768], f32, name="bF")
                nc.sync.dma_start(out=bF, in_=mrgF[b])
                bH = batchp.tile([H, 2, 512], f16, name="bH")
                nc.sync.dma_start(out=bH, in_=mrgH[b])
                outt0 = outp.tile([H, 8 * L], f32, name="outt0")
                outt1 = outp.tile([H, 8 * L], f32, name="outt1")
                batch_st[b] = dict(bF=bF, bH=bH, outts=(outt0, outt1))
                # absorb the batch-load DMA sems once on DVE and PE
                scr = apool.tile([H, 2], f32, name="dve_scr")
                nc.vector.tensor_copy(scr[:, 0:1], bF[:, 0, 0:1])
                nc.vector.tensor_copy(scr[:, 1:2].bitcast(f16)[:, 0:1], bH[:, 0, 0:1])
                abF = nc.tensor.ldweights(bF[:, 0, 0:1].bitcast(bf16))
                abH = nc.tensor.ldweights(bH[:, 0, 0:1])
                dep(abF, last_pe_absorb[0], "pe absorb chain")
                dep(abH, abF, "pe absorb chain")
                last_pe_absorb[0] = abH
            bs = batch_st[b]
            bF, bH = bs["bF"], bs["bH"]
            v1Tf = bF[:, d, 0:256]
            v2Tf = bF[:, d, 256:512]
            w2d = w2_sb[:, d, :]

            # squares of v1|v2 in one Pool op
            sq = apool.tile([H, 512], f32, name="sq")
            nc.gpsimd.tensor_tensor(sq, bF[:, d, 0:512], bF[:, d, 0:512], op=OP.mult)

            pa = psA_p.tile([H, 512], f32, name="pa")
            # [0:80 n1-c0 | 80:160 n1-c1 | 160:162 v1ones (c0,c1) | 162:164 v2ones
            #  | 164:184 n2a-rep | 184:204 n2a row (p0) | 204:244 num1 (c0,c1)
            #  | 244:246 Gr | n2bT at [0:20,256:512] | idxT at [0:1,256:512] later ]
            mms = [
                nc.tensor.matmul(pa[:, 0:80], sq[:, 0:H], w2d[:, 0:80], start=True, stop=True),
                nc.tensor.matmul(pa[:, 80:160], sq[:, H:S], w2d[:, 0:80], start=True, stop=True),
                nc.tensor.matmul(pa[:, 160:161], sq[:, 0:H], w2d[:, 80:81], start=True, stop=True),
                nc.tensor.matmul(pa[:, 161:162], sq[:, H:S], w2d[:, 80:81], start=True, stop=True),
                nc.tensor.matmul(pa[:, 162:163], sq[:, 256:384], w2d[:, 80:81], start=True, stop=True),
                nc.tensor.matmul(pa[:, 163:164], sq[:, 384:512], w2d[:, 80:81], start=True, stop=True),
                nc.tensor.matmul(pa[0:20, 256:512], w2d[:, 20:40], sq[:, 256:512], start=True, stop=True),
            ]
            if first:
                for mm in mms:
                    dep(mm, ab_id, "consts before first mms")

            # m1 numerators (before the fused rsqrt: n2a-rep joins rsa's recip)
            tcol = v2Tf[:, 255:256] if d == 0 else v2Tf[:, 0:1]
            sqt = apool.tile([H, 1], f32, name="sqt")
            nc.vector.tensor_tensor(sqt, tcol, tcol, op=OP.mult)
            rhs1 = apool.tile([H, 20], f32, name="rhs1")
            nc.vector.tensor_scalar_mul(rhs1, w2d[:, 0:20], tcol)
            nc.tensor.matmul(pa[0:1, 184:204], sqt, w2d[:, 0:20], start=True, stop=True)
            n2a_sb = apool.tile([1, 20], f32, name="n2a_sb")
            nc.scalar.copy(n2a_sb, pa[0:1, 184:204])
            nc.tensor.matmul(pa[:, 164:184], onesr_sb, n2a_sb, start=True, stop=True)
            nc.tensor.matmul(pa[:, 204:224], v1Tf[:, 0:H], rhs1, start=True, stop=True)
            nc.tensor.matmul(pa[:, 224:244], v1Tf[:, H:S], rhs1, start=True, stop=True)

            # fused rsqrt of n1 + ones-norms + rs2rep: one reciprocal + one sqrt
            rsa = apool.tile([H, 184], f32, name="rsa")
            nc.vector.reciprocal(rsa, pa[:, 0:184])
            nc.scalar.sqrt(rsa, rsa)
            # rs2 for m2: [20,256] rsqrt -> f16 -> DRAM bounce broadcast
            r2bT = apool.tile([20, 256], f32, name="r2bT")
            nc.vector.reciprocal(r2bT, pa[0:20, 256:512])
            r2bTb = apool.tile([20, 256], f16, name="r2bTb")
            nc.scalar.sqrt(r2bTb, r2bT)
            nc.sync.dma_start(
                out=r2scr[b, d].rearrange("o (l s) -> (o l) s", l=L), in_=r2bTb[:]
            )
            repb = repp.tile([H, L, S], f16, name="repb")
            nc.sync.dma_start(
                out=repb[:].rearrange("h l s -> h (l s)"),
                in_=r2scr[b, d].to_broadcast((H, L * S)),
            )

            # m2 stream prep: v2wsb = v2 * w2 per perspective, on the (idle)
            # Pool engine via stride-0-broadcast TTs; *rs2 happens in C on DVE.
            # Emitted 2 rounds ahead of use so the Pool WAR on pair i-3's m2
            # matmuls never stalls the pipeline.
            v2wsb = v2wsp.tile([H, L, S], f16, name="v2wsb")
            v2Tb_b = bH[:, d, 256:512].rearrange("h q -> h () q").to_broadcast((H, 4, S))
            for j in range(5):
                nc.gpsimd.tensor_tensor(
                    v2wsb[:, 4 * j : 4 * j + 4, :], v2Tb_b, w2rep[:, d, 4 * j : 4 * j + 4, :], op=OP.mult
                )
            pe_tickle()

            # eps * ||v1|| per chunk (for the m3 sign), both chunks at once
            nv1e = apool.tile([H, 2], f32, name="nv1e")
            nc.scalar.sqrt(nv1e, pa[:, 160:162])
            nc.scalar.mul(nv1e, nv1e, EPS)

            st[i].update(
                pa=pa, rsa=rsa, repb=repb, nv1e=nv1e, v2wsb=v2wsb,
                v1Tf=v1Tf, v2Tf=v2Tf,
                v1Tb=bH[:, d, 0:256], v2Tb=bH[:, d, 256:512],
                v2nat=bF[:, d, 512:768].rearrange("p (c h) -> p c h", c=2),
                outts=bs["outts"], b=b, d=d,
            )

        def emit_B(i):
            s = st[i]
            b, d = s["b"], s["d"]
            pa, rsa = s["pa"], s["rsa"]
            v1Tf, v2Tf, v2nat = s["v1Tf"], s["v2Tf"], s["v2nat"]
            outts = s["outts"]

            pb = psB_p.tile([H, 512], f32, name="pb")
            # GT[q, (c p)] = v2^T v1
            nc.tensor.matmul(pb[:, 0:256], v2Tf[:, 0:H], v1Tf, start=True, stop=True)
            nc.tensor.matmul(pb[:, 256:512], v2Tf[:, H:S], v1Tf, start=True, stop=True)
            GT_sb = bpool.tile([H, 2, S], f32, name="GT_sb")
            nc.scalar.copy(GT_sb[:].rearrange("h c q -> h (c q)"), pb[:, 0:512])
            pe_tickle()

            # v2r = v2 rows * rs2 (per-q-partition scale on ACT)
            v2r = bpool.tile([H, 2, H], f32, name="v2r")
            nc.scalar.activation(v2r[:, 0, :], v2nat[:, 0, :], AF.Copy, scale=rsa[:, 162:163])
            nc.scalar.activation(v2r[:, 1, :], v2nat[:, 1, :], AF.Copy, scale=rsa[:, 163:164])

            # GWT (mean-attentive, scale-invariant) at pb[0:256]; v2r^T at [256:512]
            nc.tensor.matmul(pb[:, 0:256], v2r[:, 0, :], GT_sb[:, 0, :], start=True, stop=False)
            nc.tensor.matmul(pb[:, 0:256], v2r[:, 1, :], GT_sb[:, 1, :], start=False, stop=True)
            tr0 = nc.tensor.transpose(pb[:, 256:384], v2r[:, 0, :], ident_sb)
            tr1 = nc.tensor.transpose(pb[:, 384:512], v2r[:, 1, :], ident_sb)
            if i == 0:
                dep(tr0, ab_id, "ident absorbed before transpose")
                dep(tr1, ab_id, "ident absorbed before transpose")
            v2nT = bpool.tile([H, S], f32, name="v2nT")
            nc.scalar.copy(v2nT, pb[:, 256:512])
            prod3 = bpool.tile([H, S], f32, name="prod3")
            nc.vector.tensor_tensor(prod3, v1Tf, pb[:, 0:256], op=OP.mult)
            sq3 = bpool.tile([H, S], f32, name="sq3")
            nc.scalar.square(sq3, pb[:, 0:256])

            # G' = v1 . v2n for argmax (rs1[p] scale drops out)
            nc.tensor.matmul(pb[:, 0:256], v1Tf[:, 0:H], v2nT, start=True, stop=True)
            nc.tensor.matmul(pb[:, 256:512], v1Tf[:, H:S], v2nT, start=True, stop=True)
            idxf = bpool.tile([H, 2], f32, name="idxf")
            idx8 = bpool.tile([H, 2, 8], mybir.dt.uint32, name="idx8")
            for c in range(2):
                top8 = bpool.tile([H, 8], f32, name="top8")
                nc.vector.max_with_indices(top8, idx8[:, c, :], pb[:, 256 * c : 256 * c + 256])
            nc.vector.tensor_copy(idxf, idx8[:, :, 0:1].rearrange("p c o -> p (c o)"))
            for c in range(2):
                tr = nc.tensor.transpose(
                    pa[0:1, 256 + c * H : 256 + c * H + H], idxf[:, c : c + 1], ident_sb
                )
                if i == 0:
                    dep(tr, ab_id, "ident absorbed before transpose")
            idxT_sb = bpool.tile([1, 256], f16, name="idxT_sb")
            nc.scalar.copy(idxT_sb, pa[0:1, 256:512])
            nc.sync.dma_start(out=idxscr[b, d], in_=idxT_sb[:])
            idxrepb = bpool.tile([H, 256], f16, name="idxrepb")
            nc.sync.dma_start(out=idxrepb, in_=idxscr[b, d].to_broadcast((H, S)))
            pe_tickle()

            # m3 sign: Gr = sum_q num[q,p]*rs2[q], sgn = Sign(Gr + eps||v1||)
            for c in range(2):
                nc.tensor.matmul(pa[:, 244 + c : 245 + c], GT_sb[:, 0, c * H : c * H + H],
                                 rsa[:, 162:163], start=True, stop=False)
                nc.tensor.matmul(pa[:, 244 + c : 245 + c], GT_sb[:, 1, c * H : c * H + H],
                                 rsa[:, 163:164], start=False, stop=True)
            sgn = bpool.tile([H, 2], f32, name="sgn")
            nc.scalar.activation(sgn[:, 0:1], pa[:, 244:245], AF.Sign, bias=s["nv1e"][:, 0:1], scale=1.0)
            nc.scalar.activation(sgn[:, 1:2], pa[:, 245:246], AF.Sign, bias=s["nv1e"][:, 1:2], scale=1.0)

            # m1 finals
            t1b = bpool.tile([H, 2, 20], f32, name="t1b")
            nc.vector.tensor_tensor(
                t1b,
                pa[:, 204:244].rearrange("p (c x) -> p c x", c=2),
                rsa[:, 0:160].rearrange("p (c x) -> p c x", c=2)[:, :, 0:20],
                op=OP.mult,
            )
            for c in range(2):
                nc.vector.tensor_tensor(
                    outts[c][:, d * 20 : d * 20 + 20], t1b[:, c, :], rsa[:, 164:184], op=OP.mult
                )
            s.update(GT_sb=GT_sb, prod3=prod3, sq3=sq3, idxrepb=idxrepb, sgn=sgn)

        def emit_C(i):
            s = st[i]
            b, d = s["b"], s["d"]
            pa, rsa, repb = s["pa"], s["rsa"], s["repb"]
            v1Tf, v1Tb = s["v1Tf"], s["v1Tb"]
            outts = s["outts"]
            w2d = w2_sb[:, d, :]

            # masks from the (landed) idx broadcast, then att4 gather + m3/m4 nums
            maskT0 = cpoolx.tile([H, 256], f16, name="maskT0")
            nc.vector.tensor_scalar(maskT0, s["idxrepb"], iota_sb[:, 0:1], None, op0=OP.is_equal)
            maskT1 = cpoolx.tile([H, 256], f16, name="maskT1")
            nc.vector.tensor_scalar(maskT1, s["idxrepb"], iota_sb[:, 1:2], None, op0=OP.is_equal)

            pc0 = psNum_p.tile([H, 4, 256], f32, name="psNum")
            pc0f = pc0[:].rearrange("p l q -> p (l q)")
            a4_mm0 = nc.tensor.matmul(pc0f[:, 160:416], q0cb[:, d, 0, :], maskT0, start=True, stop=False)
            a4_mm1 = nc.tensor.matmul(pc0f[:, 160:416], q0cb[:, d, 1, :], maskT1, start=False, stop=True)
            if i == 0:
                dep(a4_mm0, ab_id, "q0c absorbed before att4T")
                dep(a4_mm1, ab_id, "q0c absorbed before att4T")
            # pc0 layout: [0:40 num3 (c0,c1) | 40:80 num4 | 80:120 n3 | 120:160 n4
            #              | 160:416 att4T]
            prod3, sq3 = s["prod3"], s["sq3"]
            for c in range(2):
                sl = slice(c * H, c * H + H)
                nc.tensor.matmul(pc0f[:, c * 20 : c * 20 + 20], prod3[:, sl], w2d[:, 40:60], start=True, stop=True)
                nc.tensor.matmul(pc0f[:, 80 + c * 20 : 80 + c * 20 + 20], sq3[:, sl], w2d[:, 40:60], start=True, stop=True)
            prod4 = cpoolx.tile([H, S], f32, name="prod4")
            nc.vector.tensor_tensor(prod4, v1Tf, pc0f[:, 160:416], op=OP.mult)
            sq4 = cpoolx.tile([H, S], f32, name="sq4")
            nc.scalar.square(sq4, pc0f[:, 160:416])
            for c in range(2):
                sl = slice(c * H, c * H + H)
                nc.tensor.matmul(pc0f[:, 40 + c * 20 : 40 + c * 20 + 20], prod4[:, sl], w2d[:, 60:80], start=True, stop=True)
                nc.tensor.matmul(pc0f[:, 120 + c * 20 : 120 + c * 20 + 20], sq4[:, sl], w2d[:, 60:80], start=True, stop=True)

            # m3/m4 finals
            rsq34 = cpoolx.tile([H, 80], f32, name="rsq34")
            nc.vector.reciprocal(rsq34, pc0f[:, 80:160])
            nc.scalar.sqrt(rsq34, rsq34)
            t34 = cpoolx.tile([H, 2, 2, 20], f32, name="t34")  # [m34, c, l]
            for j in range(2):  # 0: m3 (w5/w6 rs1), 1: m4 (w7/w8 rs1)
                nc.vector.tensor_tensor(
                    t34[:, j],
                    pc0f[:, j * 40 : j * 40 + 40].rearrange("p (c x) -> p c x", c=2),
                    rsa[:, 0:160].rearrange("p (c x) -> p c x", c=2)[:, :, 40 + 20 * j : 60 + 20 * j],
                    op=OP.mult,
                )
            t34b = cpoolx.tile([H, 2, 2, 20], f32, name="t34b")
            nc.vector.tensor_tensor(
                t34b, t34,
                rsq34[:].rearrange("p (j c x) -> p j c x", j=2, c=2),
                op=OP.mult,
            )
            for c in range(2):
                nc.scalar.mul(
                    outts[c][:, 80 + d * 20 : 80 + d * 20 + 20], t34b[:, 0, c, :], s["sgn"][:, c : c + 1]
                )
                nc.vector.tensor_copy(
                    outts[c][:, 120 + d * 20 : 120 + d * 20 + 20], t34b[:, 1, c, :]
                )

            # ---- m2 ----
            # v2ws = (v2*w2, Pool-built in B) * rs2, one in-place fp16 TT
            v2wsb = s["v2wsb"]
            vwf = v2wsb[:].rearrange("h l q -> h (l q)")
            nc.vector.tensor_tensor(vwf, vwf, repb[:].rearrange("h l s -> h (l s)"), op=OP.mult)

            stage = m2pool.tile([H, 2, L, S], f16, name="stage")
            trt = m2pool.tile([H, 2, L, H], f16, name="trt")
            for c in range(2):
                for j in range(5):
                    pc = psNum_p.tile([H, 4, 256], f32, name="psNum")
                    pcf = pc[:].rearrange("p l q -> p (l q)")
                    mm = nc.tensor.matmul(
                        pcf[:, 0:512], v1Tb[:, c * H : c * H + H], vwf[:, 1024 * j : 1024 * j + 512],
                        start=True, stop=True,
                    )
                    nc.tensor.matmul(
                        pcf[:, 512:1024], v1Tb[:, c * H : c * H + H], vwf[:, 1024 * j + 512 : 1024 * j + 1024],
                        start=True, stop=True,
                    )
                    if i == 0 and c == 0 and j == 0:
                        dep(mm, last_pe_absorb[0], "absorbs before m2")
                    nc.scalar.copy(stage[:, c, 4 * j : 4 * j + 4, :], pc[:])
                    pe_tickle()
                # tree level 1 for this chunk right away, so the next pair's
                # drains (stage WAR, bufs=1) unblock as early as possible
                nc.vector.tensor_tensor(trt[:, c], stage[:, c, :, 0:H], stage[:, c, :, H:S], op=OP.max)

            # fp16 TT max-tree levels 2+: ping-pong trt/tr2 down to m2pre
            tr2 = m2pool.tile([H, 2, L, 64], f16, name="tr2")
            nc.vector.tensor_tensor(tr2, trt[:, :, :, 0:64], trt[:, :, :, 64:128], op=OP.max)
            nc.vector.tensor_tensor(trt[:, :, :, 0:32], tr2[:, :, :, 0:32], tr2[:, :, :, 32:64], op=OP.max)
            nc.vector.tensor_tensor(tr2[:, :, :, 0:16], trt[:, :, :, 0:16], trt[:, :, :, 16:32], op=OP.max)
            nc.vector.tensor_tensor(trt[:, :, :, 0:8], tr2[:, :, :, 0:8], tr2[:, :, :, 8:16], op=OP.max)
            nc.vector.tensor_tensor(tr2[:, :, :, 0:4], trt[:, :, :, 0:4], trt[:, :, :, 4:8], op=OP.max)
            nc.vector.tensor_tensor(trt[:, :, :, 0:2], tr2[:, :, :, 0:2], tr2[:, :, :, 2:4], op=OP.max)
            m2pre = cpoolx.tile([H, 2, L], f32, name="m2pre")
            nc.vector.tensor_tensor(
                m2pre,
                trt[:, :, :, 0:1].rearrange("p c l o -> p c (l o)"),
                trt[:, :, :, 1:2].rearrange("p c l o -> p c (l o)"),
                op=OP.max,
            )
            for c in range(2):
                nc.vector.tensor_tensor(
                    outts[c][:, 40 + d * 20 : 40 + d * 20 + 20],
                    m2pre[:, c, :],
                    rsa[:, 80 * c + 20 : 80 * c + 40],
                    op=OP.mult,
                )
            if d == 1:
                nc.sync.dma_start(out=out[b, 0:H, :], in_=outts[0])
                nc.sync.dma_start(out=out[b, H:S, :], in_=outts[1])

        for r in range(NP + 2):
            if r < NP:
                emit_A(r)
            if 1 <= r <= NP:
                emit_B(r - 1)
            if r >= 2:
                emit_C(r - 2)

    return nc


def _prep_core_inputs(p, q, w_list, core):
    """Host-side layout prep for one core. Only layout transforms + weight-only math."""
    sl = slice(core * BPC, (core + 1) * BPC)
    p8 = np.ascontiguousarray(p[sl])  # (BPC, 256, 256)
    q8 = np.ascontiguousarray(q[sl])
    # [b, h, d, 0:256]=pT, [256:512]=qT, [512:768]=qn rows (c,h)
    pT = p8.reshape(BPC, S, 2, H).transpose(0, 3, 2, 1)  # (BPC, H, 2, S)
    qT = q8.reshape(BPC, S, 2, H).transpose(0, 3, 2, 1)
    # qn[b, qp, d, c, h] = q8[b, c*128+qp, d*H+h]
    qn = q8.reshape(BPC, 2, H, 2, H).transpose(0, 2, 3, 1, 4)  # (BPC, qp, d, c, h)
    mrgF = np.empty((BPC, H, 2, 768), np.float32)
    mrgF[..., 0:256] = pT
    mrgF[..., 256:512] = qT
    mrgF[..., 512:768] = qn.reshape(BPC, H, 2, 256)
    mrgH = np.empty((BPC, H, 2, 512), np.float16)
    mrgH[..., 0:256] = pT
    mrgH[..., 256:512] = qT

    q0n = np.ascontiguousarray(q[0].reshape(S, 2, H).transpose(1, 0, 2))  # (2, S, H)

    w2T = np.empty((2, H, 81), np.float32)
    for d in range(2):
        ws = w_list[d::2]  # fw: w1,w3,w5,w7 ; bw: w2,w4,w6,w8
        cat = np.concatenate([w * w for w in ws] + [np.ones((1, H), np.float32)], 0)
        w2T[d] = cat.T
    # w2rep[d, h, l*S+s] = (w3/w4)^2[l, h] replicated over s
    w2rep = np.empty((2, H, L * S), np.float16)
    for d in range(2):
        w2 = (w_list[2 + d].astype(np.float32) ** 2).astype(np.float16)  # (L, H)
        w2rep[d] = np.repeat(w2.T[:, :, None], S, axis=2).reshape(H, L * S)
    iota2 = np.stack([np.arange(H, dtype=np.float32), np.arange(H, 2 * H, dtype=np.float32)], 1)

    return {
        "mrgF": mrgF,
        "mrgH": mrgH,
        "w2T": w2T,
        "w2repI": w2rep,
        "q0nb": q0n.astype(np.float16),
        "iota2": np.ascontiguousarray(iota2),
        "onesr": np.ones((1, H), np.float32),
        "ident": np.eye(H, dtype=np.float32),
    }


def _legalize_bir(bir_bytes):
    """This walrus build rejects >1 sync-wait command per instruction; move all
    but one wait of each instruction onto an inserted same-engine Drain."""
    import json as _json

    d = _json.loads(bir_bytes)
    n = 0
    for fnd in d["functions"]:
        for blk in fnd["blocks"]:
            insts = blk.get("instructions") or []
            out = []
            for ins in insts:
                si = ins.get("sync_info") or {}
                w = si.get("on_wait") or []
                if len(w) > 1:
                    for extra in w[:-1]:
                        out.append(
                            {
                                "debug": ins.get("debug", 0),
                                "engine": ins.get("engine"),
                                "ins": [],
                                "outs": [],
                                "is_reset_sema": False,
                                "name": f"I-legalw-{n}",
                                "opcode": "Drain",
                                "sync_info": {"on_update": [], "on_wait": [extra]},
                            }
                        )
                        n += 1
                    si["on_wait"] = [w[-1]]
                out.append(ins)
            blk["instructions"] = out
    return _json.dumps(d).encode(), n


def _install_legalizer():
    if _cache.get("legalizer"):
        return
    from concourse import bass2jax, bass_utils

    orig = bass_utils.compile_bir_kernel

    def patched(bir_json, tmpdir, neff_name="file.neff"):
        fixed, n = _legalize_bir(bir_json)
        return orig(fixed, tmpdir, neff_name)

    bass2jax.compile_bir_kernel = patched
    _cache["legalizer"] = True


def _get_runner():
    """Build the 8-core shard_map'd PJRT callable once."""
    if "runner" in _cache:
        return _cache["runner"]

    import jax
    from jax.sharding import Mesh, PartitionSpec
    from jax.experimental.shard_map import shard_map

    import concourse.mybir as mybir
    from concourse import bass2jax

    if "nc" not in _cache:
        _cache["nc"] = _build_bass()
    nc = _cache["nc"]

    bass2jax.install_neuronx_cc_hook()
    _install_legalizer()
    assert nc.dbg_addr is None
    partition_name = nc.partition_id_tensor.name if nc.partition_id_tensor else None

    in_names, out_names, out_avals, zero_outs = [], [], [], []
    for alloc in nc.m.functions[0].allocations:
        if not isinstance(alloc, mybir.MemoryLocationSet):
            continue
        name = alloc.memorylocations[0].name
        if alloc.kind == "ExternalInput":
            if name != partition_name:
                in_names.append(name)
        elif alloc.kind == "ExternalOutput":
            out_names.append(name)
            shape = tuple(alloc.tensor_shape)
            dtype = mybir.dt.np(alloc.dtype)
            out_avals.append(jax.core.ShapedArray(shape, dtype))
            zero_outs.append(np.zeros(shape, dtype))
    n_params = len(in_names)
    n_outs = len(out_avals)
    all_names = in_names + out_names
    if partition_name is not None:
        all_names = all_names + [partition_name]

    def _body(*args):
        operands = list(args)
        if partition_name is not None:
            operands.append(bass2jax.partition_id_tensor())
        outs = bass2jax._bass_exec_p.bind(
            *operands,
            out_avals=tuple(out_avals),
            in_names=tuple(all_names),
            out_names=tuple(out_names),
            lowering_input_output_aliases=(),
            sim_require_finite=True,
            sim_require_nnan=True,
            nc=nc,
        )
        return tuple(outs)

    devices = jax.devices()[:NCORES]
    mesh = Mesh(np.asarray(devices), ("core",))
    sharded = jax.jit(
        shard_map(
            _body,
            mesh=mesh,
            in_specs=(PartitionSpec("core"),) * (n_params + n_outs),
            out_specs=(PartitionSpec("core"),) * n_outs,
            check_rep=False,
        ),
        donate_argnums=tuple(range(n_params, n_params + n_outs)),
        keep_unused=True,
    )
    runner = {
        "jax": jax,
        "sharded": sharded,
        "mesh": mesh,
        "in_names": in_names,
        "out_names": out_names,
        "out_avals": out_avals,
        "zero_outs": zero_outs,
        "n_params": n_params,
    }
    _cache["runner"] = runner
    return runner


def kernel(p, q, w1, w2, w3, w4, w5, w6, w7, w8, _time_iters=0):
    p = np.asarray(p, dtype=np.float32)
    q = np.asarray(q, dtype=np.float32)
    w_list = [np.asarray(w, dtype=np.float32) for w in (w1, w2, w3, w4, w5, w6, w7, w8)]

    r = _get_runner()
    jax = r["jax"]
    in_maps = [_prep_core_inputs(p, q, w_list, c) for c in range(NCORES)]
    concat_in = [
        np.concatenate([in_maps[c][name] for c in range(NCORES)], 0)
        for name in r["in_names"]
    ]
    concat_zeros = [
        np.zeros((NCORES * z.shape[0], *z.shape[1:]), z.dtype) for z in r["zero_outs"]
    ]
    out_arrs = r["sharded"](*concat_in, *concat_zeros)
    jax.block_until_ready(out_arrs)
    out = np.asarray(out_arrs[r["out_names"].index("out")])  # (64, 256, 160)

    if _time_iters:
        import time

        from jax.sharding import NamedSharding, PartitionSpec

        shd = NamedSharding(r["mesh"], PartitionSpec("core"))
        dev_in = [jax.device_put(a, shd) for a in concat_in]
        jax.block_until_ready(dev_in)
        times = []
        for _ in range(_time_iters):
            zeros = [
                jax.device_put(np.zeros((NCORES * z.shape[0], *z.shape[1:]), z.dtype), shd)
                for z in r["zero_outs"]
            ]
            jax.block_until_ready(zeros)
            t0 = time.perf_counter()
            o = r["sharded"](*dev_in, *zeros)
            jax.block_until_ready(o)
            times.append(time.perf_counter() - t0)
        kernel.last_exec_time_ns = int(min(times) * 1e9)
        kernel.all_times_ns = [int(t * 1e9) for t in times]
    return out
